# revision 1
# baseline (speedup 1.0000x reference)
"""Trainium2 Bass kernel: multi-head self-attention (B=2, T=2048, D=1024, H=16),
8-core SPMD. Accepts FULL inputs, returns the FULL output.

Sharding: data-parallel over batch (2) x tensor-parallel over heads (4 groups
of 4). Each core computes attention for its 4 heads of one batch plus its
partial output projection; the host sums the 4 partials per batch (plus the
bias terms, folded exactly). All matmuls run in float32r (TF32) on the PE.
"""
import sys
if '/opt/trn_rl_repo' not in sys.path:
    sys.path.insert(0, '/opt/trn_rl_repo')
import numpy as np
import concourse.bass as bass
import concourse.mybir as mybir
from concourse import bacc
from concourse.tile import TileContext

F32 = mybir.dt.float32
F32R = mybir.dt.float32r
AL = mybir.AluOpType
EXP = mybir.ActivationFunctionType.Exp

T = 2048
DM = 1024
HPC = 4
D = 64
NQB = 4           # query blocks of 512
NKC = 16          # key chunks of 128
NDC = 8           # contraction chunks of 128 for projections
LAG = 3           # PV lags S/exp by this many key chunks


def build_nc():
    nc = bacc.Bacc("TRN2", target_bir_lowering=False, debug=True)

    xT = nc.dram_tensor("xT", [DM, T], F32R, kind="ExternalInput")
    wq = nc.dram_tensor("wq", [DM, 256], F32R, kind="ExternalInput")
    wk = nc.dram_tensor("wk", [DM, 256], F32R, kind="ExternalInput")
    wv = nc.dram_tensor("wv", [DM, 260], F32R, kind="ExternalInput")
    wp = nc.dram_tensor("wp", [2, 128, DM], F32R, kind="ExternalInput")
    msk = nc.dram_tensor("msk", [NQB, 128, 1024], F32, kind="ExternalInput")
    y = nc.dram_tensor("y", [T, DM], F32, kind="ExternalOutput")

    with nc.allow_low_precision("tf32 matmul pipeline"), TileContext(nc) as tc:
        from contextlib import ExitStack
        ctx = ExitStack()
        cp = ctx.enter_context(tc.tile_pool(name="const", bufs=1))
        wtp = ctx.enter_context(tc.tile_pool(name="wts", bufs=1))
        qkvp = ctx.enter_context(tc.tile_pool(name="qkv", bufs=1))
        psS = ctx.enter_context(tc.tile_pool(name="psS", bufs=2, space="PSUM"))
        psO = ctx.enter_context(tc.tile_pool(name="psO", bufs=1, space="PSUM"))
        psX = ctx.enter_context(tc.tile_pool(name="psX", bufs=2, space="PSUM"))

        # constants: duplicated causal masks (same pattern for both heads)
        # (mask DMAs are emitted after the xT/weight DMAs below - phase C
        # needs them much later)
        mask_t = [cp.tile([128, 1024], F32, tag=f"m{i}", name=f"m{i}")
                  for i in range(NQB)]
        ones32 = cp.tile([128, 64], F32, tag="ones32", name="ones32")
        nc.vector.memset(ones32[:], 1.0)
        ones_t = cp.tile([128, 64], F32R, tag="ones", name="ones")
        nc.vector.tensor_copy(ones_t[:], ones32[:])
        onesv = cp.tile([128, 4], F32, tag="onesv", name="onesv")
        nc.vector.memset(onesv[:], 1.0)

        # weights
        wq_t = [wtp.tile([128, 256], F32R, tag=f"wq{k}", name=f"wq{k}")
                for k in range(NDC)]
        wk_t = [wtp.tile([128, 256], F32R, tag=f"wk{k}", name=f"wk{k}")
                for k in range(NDC)]
        wv_t = [wtp.tile([128, 260], F32R, tag=f"wv{k}", name=f"wv{k}")
                for k in range(NDC)]
        wp_t = [wtp.tile([128, DM], F32R, tag=f"wp{j}", name=f"wp{j}")
                for j in range(2)]

        # persistent activations
        QT = [qkvp.tile([128, T], F32R, tag=f"qt{i}", name=f"qt{i}") for i in range(2)]
        KT = [qkvp.tile([128, T], F32R, tag=f"kt{i}", name=f"kt{i}") for i in range(2)]
        V = [qkvp.tile([128, 260], F32R, tag=f"v{t}", name=f"v{t}") for t in range(NKC)]
        # stacked O^T: feature-chunk jc holds heads (2jc, 2jc+1) on
        # partitions 0-63 / 64-127
        OTS = [qkvp.tile([128, T], F32R, tag=f"ots{j}", name=f"ots{j}")
               for j in range(2)]

        # ---------------- phase B: QKV projections ----------------
        with tc.tile_pool(name="xt", bufs=1) as xtp:
            xt_t = [xtp.tile([128, T], F32R, tag=f"x{k}", name=f"x{k}")
                    for k in range(NDC)]
            for k in range(NDC):
                nc.sync.dma_start(xt_t[k][:], xT[k * 128:(k + 1) * 128, :])
            for k in range(NDC):
                nc.sync.dma_start(wq_t[k][:], wq[k * 128:(k + 1) * 128, :])
                nc.sync.dma_start(wk_t[k][:], wk[k * 128:(k + 1) * 128, :])
                nc.sync.dma_start(wv_t[k][:], wv[k * 128:(k + 1) * 128, :])
            for j in range(2):
                nc.sync.dma_start(wp_t[j][:], wp[j])
            for i in range(NQB):
                nc.sync.dma_start(mask_t[i][:], msk[i])
            for W, OUT in ((wq_t, QT), (wk_t, KT)):
                for fc in range(2):
                    for tbp in range(2):
                        ps = psS.tile([128, 1024], F32, tag="s", name="s")
                        for j in (0, 1):
                            tb = 2 * tbp + j
                            for k in range(NDC):
                                nc.tensor.matmul(
                                    ps[:, j * 512:(j + 1) * 512],
                                    W[k][:, fc * 128:(fc + 1) * 128],
                                    xt_t[k][:, tb * 512:(tb + 1) * 512],
                                    start=(k == 0), stop=(k == NDC - 1))
                        nc.scalar.copy(
                            OUT[fc][:, tbp * 1024:(tbp + 1) * 1024], ps[:])
            for tt in range(NKC):
                ps = psX.tile([128, 260], F32, tag="b", name="b")
                for k in range(NDC):
                    nc.tensor.matmul(
                        ps[:], xt_t[k][:, tt * 128:(tt + 1) * 128], wv_t[k][:],
                        start=(k == 0), stop=(k == NDC - 1))
                nc.scalar.copy(V[tt][:], ps[:])
                nc.vector.tensor_copy(
                    V[tt].rearrange("p (h c) -> p h c", c=65)[:, :, 64:65],
                    onesv[:].rearrange("p (h c) -> p h c", c=1))

        # ---------------- phase C: attention ----------------
        ptp = ctx.enter_context(tc.tile_pool(name="pt", bufs=5))
        rcp = ctx.enter_context(tc.tile_pool(name="rcp", bufs=2))
        if True:

            def make_norm_steps(hp, heads, dh, ou_all):
                """Closures: recip_step(h, half) covers query blocks
                2*half, 2*half+1; norm_step(h, qb) does the K=1 broadcast
                matmul + normalize multiply. Callers sprinkle these through
                dense K=128 matmul streams so HAM row activity stays high."""
                dhrr = {}

                def recip_step(h, half):
                    def emit():
                        hh = h % 2
                        dhr = rcp.tile([128, 512], F32, tag=f"dhr{hh}",
                                       name=f"dhr{hh}", bufs=2)
                        nc.vector.reciprocal_approx_fast(dhr[:], dh[(h, half)][:])
                        dd = rcp.tile([128, 512], F32R, tag=f"dhrr{hh}",
                                      name=f"dhrr{hh}", bufs=2)
                        nc.vector.tensor_copy(dd[:], dhr[:])
                        dhrr[(h, half)] = dd
                    return emit

                def norm_step(h, qb):
                    def emit():
                        hh = h % 2
                        r = 32 * (qb % 2)
                        dd = dhrr[(h, qb // 2)]
                        psb = psX.tile([64, 512], F32, tag="b", name="b")
                        nc.tensor.matmul(
                            psb[:], ones_t[r:r + 1, 0:64], dd[r:r + 1, :],
                            start=True, stop=True)
                        if hh == 0:
                            nc.vector.tensor_tensor(
                                OTS[hp][0:64, qb * 512:(qb + 1) * 512],
                                ou_all[(h, qb)][0:64, :], psb[:], AL.mult)
                        else:
                            # odd head: normalize to a bounce tile, DMA to
                            # partitions 64-127 of the stacked O^T
                            ob = rcp.tile([64, 512], F32R, tag="ob",
                                          name="ob", bufs=2)
                            nc.vector.tensor_tensor(
                                ob[:], ou_all[(h, qb)][0:64, :], psb[:],
                                AL.mult)
                            nc.sync.dma_start(
                                OTS[hp][64:128, qb * 512:(qb + 1) * 512],
                                ob[:])
                    return emit

                return recip_step, norm_step

            def proj_tile(tt, mb, ybp):
                psy = psX.tile([128, 512], F32, tag="b", name="yps")
                for jc in range(2):
                    nc.tensor.matmul(
                        psy[:],
                        OTS[jc][:, tt * 128:(tt + 1) * 128],
                        wp_t[jc][:, mb * 512:(mb + 1) * 512],
                        start=(jc == 0), stop=(jc == 1))
                yt = ybp.tile([128, 512], F32, tag="yt", name="yt")
                if (tt + mb) % 2 == 0:
                    nc.scalar.copy(yt[:], psy[:])
                else:
                    nc.vector.tensor_copy(yt[:], psy[:])
                nc.sync.dma_start(
                    y[tt * 128:(tt + 1) * 128, mb * 512:(mb + 1) * 512],
                    yt[:])

            ybp = ctx.enter_context(tc.tile_pool(name="yb", bufs=3))
            inject = []      # closures to sprinkle, one per key-chunk slot
            for hp in range(HPC // 2):
                fc = hp
                heads = (2 * hp, 2 * hp + 1)
                dh = {(h, half): rcp.tile([128, 512], F32,
                                          tag=f"dh{hp}_{h % 2}_{half}",
                                          name=f"dh{hp}_{h % 2}_{half}", bufs=1)
                      for h in heads for half in range(2)}
                recip_step, norm_step = None, None
                ou_all = {}
                for qb in range(NQB):
                    nkc = 4 * (qb + 1)
                    pso = {h: psO.tile([65, 512], F32, tag=f"o{h % 2}",
                                       name=f"o{h % 2}") for h in heads}
                    ptq = {}
                    for kc in range(nkc + LAG):
                        if kc < nkc:
                            # both heads' S^T for this key chunk in one PSUM
                            # tile; the two K=64 matmuls alternate PE row
                            # groups and run concurrently
                            pss = psS.tile([128, 1024], F32, tag="s", name="s")
                            for h in heads:
                                po = 64 * (h % 2)
                                nc.tensor.matmul(
                                    pss[:, po * 8:po * 8 + 512],
                                    KT[fc][po:po + 64, kc * 128:(kc + 1) * 128],
                                    QT[fc][po:po + 64, qb * 512:(qb + 1) * 512],
                                    start=True, stop=True)
                            pt = ptp.tile([128, 1024], F32R, tag="pt", name="pt")
                            nc.scalar.activation(pt[:], pss[:], EXP)
                            if kc >= 4 * qb:  # diagonal chunk -> causal mask
                                nc.vector.tensor_tensor(
                                    pt[:], pt[:], mask_t[kc - 4 * qb][:], AL.mult)
                            ptq[kc] = pt
                        kcp = kc - LAG
                        if kcp >= 0 and kcp in ptq:
                            ptv = ptq.pop(kcp)
                            for h in heads:
                                po = 64 * (h % 2)
                                nc.tensor.matmul(
                                    pso[h][:],
                                    V[kcp][:, 65 * h:65 * h + 65],
                                    ptv[:, po * 8:po * 8 + 512],
                                    start=(kcp == 0),
                                    stop=(kcp == nkc - 1))
                        if kcp >= 0 and inject:
                            inject.pop(0)()
                    for h in heads:
                        ou = rcp.tile([65, 512], F32,
                                      tag=f"ou{hp}_{h % 2}_{qb}",
                                      name=f"ou{hp}_{h % 2}_{qb}", bufs=1)
                        nc.vector.tensor_copy(ou[:], pso[h][:])
                        nc.sync.dma_start(
                            dh[(h, qb // 2)][32 * (qb % 2):32 * (qb % 2) + 1, :],
                            ou[64:65, :])
                        ou_all[(h, qb)] = ou
                    if hp == 0 and qb == NQB - 1:
                        # hp0's normalize work runs during hp1's attention
                        recip_step, norm_step = make_norm_steps(
                            hp, heads, dh, ou_all)
                        inject += [recip_step(h, half)
                                   for h in heads for half in range(2)]
                        inject += [norm_step(h, q)
                                   for q in range(NQB) for h in heads]
                    if hp == 1:
                        recip_step, norm_step = (recip_step, norm_step) \
                            if recip_step else make_norm_steps(hp, heads, dh, ou_all)
                        if qb == 1:
                            # first-half reciprocals + qb0/1 normalize + the
                            # projection tiles they unblock, all sprinkled
                            # through qb2/qb3 attention
                            inject += [recip_step(h, 0) for h in heads]
                            inject += [norm_step(h, q)
                                       for q in (0, 1) for h in heads]
                            inject += [(lambda t=t, m=m: proj_tile(t, m, ybp))
                                       for t in range(8) for m in range(2)]
                        if qb == NQB - 1:
                            inject += [recip_step(h, 1) for h in heads]
                            inject += [norm_step(h, q)
                                       for q in (2, 3) for h in heads]
                            inject += [(lambda t=t, m=m: proj_tile(t, m, ybp))
                                       for t in range(8, NKC) for m in range(2)]
            # drain any remaining injected steps (tail of the kernel)
            while inject:
                inject.pop(0)()
        ctx.close()

    nc.finalize()
    return nc


def make_masks():
    """[NQB, 128, 1024]: mask for diagonal chunk offset t, duplicated for the
    two heads (cols 0-511 and 512-1023 identical).
    keep iff query_in_block >= key_in_chunk + 128*t."""
    m = np.zeros((NQB, 128, 1024), dtype=np.float32)
    f = np.arange(512)
    p = np.arange(128)
    for t in range(NQB):
        pat = (f[None, :] >= p[:, None] + 128 * t).astype(np.float32)
        m[t][:, 0:512] = pat
        m[t][:, 512:1024] = pat
    return m


def shard_inputs(x, Wqkv, bqkv, Wproj):
    x = np.asarray(x, dtype=np.float32)
    Wqkv = np.asarray(Wqkv, dtype=np.float32)
    bqkv = np.asarray(bqkv, dtype=np.float32)
    Wproj = np.asarray(Wproj, dtype=np.float32)
    assert not np.any(bqkv[0:2048]), \
        "nonzero q/k bias not supported by the fast kernel"
    masks = make_masks()
    in_maps = []
    for c in range(8):
        b, g = c // 4, c % 4
        cs = slice(256 * g, 256 * g + 256)
        wq_ = np.ascontiguousarray(Wqkv[:, 0:1024][:, cs]) / 8.0
        wk_ = np.ascontiguousarray(Wqkv[:, 1024:2048][:, cs])
        wv_src = Wqkv[:, 2048:3072][:, cs]
        wv_ = np.zeros((DM, 260), dtype=np.float32)
        for h in range(4):
            wv_[:, 65 * h:65 * h + 64] = wv_src[:, 64 * h:64 * h + 64]
        wp_ = np.ascontiguousarray(
            Wproj[256 * g:256 * g + 256, :].reshape(2, 128, DM))
        in_maps.append({
            "xT": np.ascontiguousarray(x[b].T),
            "wq": wq_, "wk": wk_, "wv": wv_, "wp": wp_, "msk": masks,
        })
    return in_maps


def combine_outputs(results, Wqkv, bqkv, Wproj, bproj):
    bqkv = np.asarray(bqkv, dtype=np.float32)
    Wproj = np.asarray(Wproj, dtype=np.float32)
    bproj = np.asarray(bproj, dtype=np.float32)
    bv_term = bqkv[2048:3072] @ Wproj
    out = np.zeros((2, T, DM), dtype=np.float32)
    for c in range(8):
        out[c // 4] += results[c]["y"]
    out += (bv_term + bproj)[None, None, :]
    return out


_NC_CACHE = []


def _numpy_fallback(x, Wqkv, bqkv, Wproj, bproj):
    # exact-but-slow path for inputs the device kernel does not support
    b, t, dm = x.shape
    h, d = 16, 64
    qkv = x @ Wqkv + bqkv
    q, k, v = np.split(qkv, 3, axis=-1)
    q = q.reshape(b, t, h, d).transpose(0, 2, 1, 3)
    k = k.reshape(b, t, h, d).transpose(0, 2, 1, 3)
    v = v.reshape(b, t, h, d).transpose(0, 2, 1, 3)
    att = np.einsum('bhqd,bhkd->bhqk', q, k) / np.sqrt(np.float32(d))
    causal = np.tril(np.ones((t, t), dtype=bool))
    att = np.where(causal[None, None], att, -np.inf)
    att = att - att.max(axis=-1, keepdims=True)
    e = np.exp(att)
    p = e / e.sum(axis=-1, keepdims=True)
    out = np.einsum('bhqk,bhkd->bhqd', p, v)
    out = out.transpose(0, 2, 1, 3).reshape(b, t, dm)
    return (out @ Wproj + bproj).astype(np.float32)


def kernel(x, Wqkv, bqkv, Wproj, bproj):
    x = np.asarray(x, dtype=np.float32)
    Wqkv = np.asarray(Wqkv, dtype=np.float32)
    bqkv = np.asarray(bqkv, dtype=np.float32)
    Wproj = np.asarray(Wproj, dtype=np.float32)
    bproj = np.asarray(bproj, dtype=np.float32)
    if np.any(bqkv[0:2048]):
        # nonzero q/k bias falls outside the fused device kernel's contract
        return _numpy_fallback(x, Wqkv, bqkv, Wproj, bproj)
    from concourse.bass_utils import run_bass_kernel_spmd
    if not _NC_CACHE:
        _NC_CACHE.append(build_nc())
    nc = _NC_CACHE[0]
    in_maps = shard_inputs(x, Wqkv, bqkv, Wproj)
    res = run_bass_kernel_spmd(nc, in_maps, core_ids=list(range(8)))
    return combine_outputs(res.results, Wqkv, bqkv, Wproj, bproj)



# revision 2
# speedup vs baseline: 1.2601x; 1.2601x over previous
"""Trainium2 Bass kernel: multi-head self-attention (B=2, T=2048, D=1024, H=16),
8-core SPMD. Accepts FULL inputs, returns the FULL output.

Sharding: data-parallel over batch (2) x tensor-parallel over heads (4 groups
of 4). Each core computes attention for its 4 heads of one batch plus its
partial output projection; the host sums the 4 partials per batch (plus the
bias terms, folded exactly). Matmuls run in bf16 on the PE (2x the fp32r
stream rate); accumulation is fp32 in PSUM and softmax denominators stay fp32.
"""
import sys
if '/opt/trn_rl_repo' not in sys.path:
    sys.path.insert(0, '/opt/trn_rl_repo')
import numpy as np
import ml_dtypes
import concourse.bass as bass
import concourse.mybir as mybir
from concourse import bacc
from concourse.tile import TileContext

F32 = mybir.dt.float32
F32R = mybir.dt.float32r
BF16 = mybir.dt.bfloat16
AL = mybir.AluOpType
EXP = mybir.ActivationFunctionType.Exp

T = 2048
DM = 1024
HPC = 4
D = 64
NQB = 4           # query blocks of 512
NKC = 16          # key chunks of 128
NDC = 8           # contraction chunks of 128 for projections
LAG = 3           # PV lags S/exp by this many key chunks


def build_nc():
    nc = bacc.Bacc("TRN2", target_bir_lowering=False, debug=True)

    xT = nc.dram_tensor("xT", [DM, T], BF16, kind="ExternalInput")
    wq = nc.dram_tensor("wq", [DM, 256], BF16, kind="ExternalInput")
    wk = nc.dram_tensor("wk", [DM, 256], BF16, kind="ExternalInput")
    wv = nc.dram_tensor("wv", [DM, 260], BF16, kind="ExternalInput")
    wp = nc.dram_tensor("wp", [2, 128, DM], BF16, kind="ExternalInput")
    msk = nc.dram_tensor("msk", [NQB, 128, 1024], BF16, kind="ExternalInput")
    y = nc.dram_tensor("y", [T, DM], F32, kind="ExternalOutput")

    with nc.allow_low_precision("bf16 matmul pipeline"), TileContext(nc) as tc:
        from contextlib import ExitStack
        ctx = ExitStack()
        cp = ctx.enter_context(tc.tile_pool(name="const", bufs=1))
        wtp = ctx.enter_context(tc.tile_pool(name="wts", bufs=1))
        qkvp = ctx.enter_context(tc.tile_pool(name="qkv", bufs=1))
        psS = ctx.enter_context(tc.tile_pool(name="psS", bufs=2, space="PSUM"))
        psO = ctx.enter_context(tc.tile_pool(name="psO", bufs=1, space="PSUM"))
        psX = ctx.enter_context(tc.tile_pool(name="psX", bufs=2, space="PSUM"))

        # constants: duplicated causal masks (same pattern for both heads)
        # (mask DMAs are emitted after the xT/weight DMAs below - phase C
        # needs them much later)
        mask_t = [cp.tile([128, 1024], BF16, tag=f"m{i}", name=f"m{i}")
                  for i in range(NQB)]
        ones32 = cp.tile([128, 64], F32, tag="ones32", name="ones32")
        nc.vector.memset(ones32[:], 1.0)
        # preload the exp ACT table set while the input DMAs stream in
        warm = cp.tile([1, 8], F32, tag="warm", name="warm")
        nc.scalar.activation(warm[:], ones32[0:1, 0:8], EXP)
        ones_t = cp.tile([128, 64], F32R, tag="ones", name="ones")
        nc.vector.tensor_copy(ones_t[:], ones32[:])
        onesv = cp.tile([128, 4], F32, tag="onesv", name="onesv")
        nc.vector.memset(onesv[:], 1.0)

        # weights
        wq_t = [wtp.tile([128, 256], BF16, tag=f"wq{k}", name=f"wq{k}")
                for k in range(NDC)]
        wk_t = [wtp.tile([128, 256], BF16, tag=f"wk{k}", name=f"wk{k}")
                for k in range(NDC)]
        wv_t = [wtp.tile([128, 260], BF16, tag=f"wv{k}", name=f"wv{k}")
                for k in range(NDC)]
        wp_t = [wtp.tile([128, DM], BF16, tag=f"wp{j}", name=f"wp{j}")
                for j in range(2)]

        # persistent activations
        QT = [qkvp.tile([128, T], BF16, tag=f"qt{i}", name=f"qt{i}") for i in range(2)]
        KT = [qkvp.tile([128, T], BF16, tag=f"kt{i}", name=f"kt{i}") for i in range(2)]
        V = [qkvp.tile([128, 260], BF16, tag=f"v{t}", name=f"v{t}") for t in range(NKC)]
        # stacked O^T: feature-chunk jc holds heads (2jc, 2jc+1) on
        # partitions 0-63 / 64-127
        OTS = [qkvp.tile([128, T], BF16, tag=f"ots{j}", name=f"ots{j}")
               for j in range(2)]

        # ---------------- phase B: QKV projections ----------------
        with tc.tile_pool(name="xt", bufs=1) as xtp:
            xt_t = [xtp.tile([128, T], BF16, tag=f"x{k}", name=f"x{k}")
                    for k in range(NDC)]
            # DMA order matches consumption: Q needs wq + x cols 0-1023
            # first (tbp=0), then the second x halves (tbp=1), then wk, ...
            for k in range(NDC):
                nc.sync.dma_start(wq_t[k][:], wq[k * 128:(k + 1) * 128, :])
                nc.sync.dma_start(xt_t[k][:, 0:1024],
                                  xT[k * 128:(k + 1) * 128, 0:1024])
            for k in range(NDC):
                nc.sync.dma_start(wk_t[k][:], wk[k * 128:(k + 1) * 128, :])
                nc.sync.dma_start(xt_t[k][:, 1024:2048],
                                  xT[k * 128:(k + 1) * 128, 1024:2048])
            for k in range(NDC):
                nc.sync.dma_start(wv_t[k][:], wv[k * 128:(k + 1) * 128, :])
            for i in range(NQB):
                nc.sync.dma_start(mask_t[i][:], msk[i])
            for j in range(2):
                nc.sync.dma_start(wp_t[j][:], wp[j])
            ncopy = 0
            for W, OUT in ((wq_t, QT), (wk_t, KT)):
                for tbp in range(2):
                    for fc in range(2):
                        ps = psS.tile([128, 1024], F32, tag="s", name="s")
                        for j in (0, 1):
                            tb = 2 * tbp + j
                            for k in range(NDC):
                                nc.tensor.matmul(
                                    ps[:, j * 512:(j + 1) * 512],
                                    W[k][:, fc * 128:(fc + 1) * 128],
                                    xt_t[k][:, tb * 512:(tb + 1) * 512],
                                    start=(k == 0), stop=(k == NDC - 1))
                        dst = OUT[fc][:, tbp * 1024:(tbp + 1) * 1024]
                        if ncopy % 2 == 0:
                            nc.scalar.copy(dst, ps[:])
                        else:
                            nc.vector.tensor_copy(dst, ps[:])
                        ncopy += 1
            for tt in range(NKC):
                ps = psX.tile([128, 260], F32, tag="b", name="b")
                for k in range(NDC):
                    nc.tensor.matmul(
                        ps[:], xt_t[k][:, tt * 128:(tt + 1) * 128], wv_t[k][:],
                        start=(k == 0), stop=(k == NDC - 1))
                if tt % 2 == 0:
                    nc.scalar.copy(V[tt][:], ps[:])
                else:
                    nc.vector.tensor_copy(V[tt][:], ps[:])
                nc.vector.tensor_copy(
                    V[tt].rearrange("p (h c) -> p h c", c=65)[:, :, 64:65],
                    onesv[:].rearrange("p (h c) -> p h c", c=1))

        # ---------------- phase C: attention ----------------
        ptp = ctx.enter_context(tc.tile_pool(name="pt", bufs=5))
        rcp = ctx.enter_context(tc.tile_pool(name="rcp", bufs=2))
        if True:

            def make_norm_steps(hp, dh, ou_all):
                """Closures: recip_step(qb) inverts the [dh] row pair for one
                query block (head-even at partition 0, head-odd at 32);
                norm_step(h, qb) does the K=1 broadcast matmul + normalize
                multiply. Callers sprinkle these through dense K=128 matmul
                streams so HAM row activity stays high."""
                dhrr = {}

                def recip_step(qb):
                    def emit():
                        dhr = rcp.tile([64, 512], F32, tag="dhr",
                                       name="dhr", bufs=2)
                        nc.vector.reciprocal_approx_fast(dhr[:], dh[qb][0:64, :])
                        dd = rcp.tile([64, 512], F32R, tag="dhrr",
                                      name="dhrr", bufs=2)
                        nc.vector.tensor_copy(dd[:], dhr[:])
                        dhrr[qb] = dd
                    return emit

                def norm_step(h, qb):
                    def emit():
                        hh = h % 2
                        r = 32 * hh
                        dd = dhrr[qb]
                        psb = psX.tile([64, 512], F32, tag="b", name="b")
                        nc.tensor.matmul(
                            psb[:], ones_t[r:r + 1, 0:64], dd[r:r + 1, :],
                            start=True, stop=True)
                        if hh == 0:
                            nc.vector.tensor_tensor(
                                OTS[hp][0:64, qb * 512:(qb + 1) * 512],
                                ou_all[(h, qb)][0:64, :], psb[:], AL.mult)
                        else:
                            # odd head: normalize to a bounce tile, DMA to
                            # partitions 64-127 of the stacked O^T
                            ob = rcp.tile([64, 512], BF16, tag="ob",
                                          name="ob", bufs=2)
                            nc.vector.tensor_tensor(
                                ob[:], ou_all[(h, qb)][0:64, :], psb[:],
                                AL.mult)
                            nc.sync.dma_start(
                                OTS[hp][64:128, qb * 512:(qb + 1) * 512],
                                ob[:])
                    return emit

                return recip_step, norm_step

            def proj_tile(tt, mb, ybp):
                psy = psX.tile([128, 512], F32, tag="b", name="yps")
                for jc in range(2):
                    nc.tensor.matmul(
                        psy[:],
                        OTS[jc][:, tt * 128:(tt + 1) * 128],
                        wp_t[jc][:, mb * 512:(mb + 1) * 512],
                        start=(jc == 0), stop=(jc == 1))
                yt = ybp.tile([128, 512], F32, tag="yt", name="yt")
                if (tt + mb) % 2 == 0:
                    nc.scalar.copy(yt[:], psy[:])
                else:
                    nc.vector.tensor_copy(yt[:], psy[:])
                nc.sync.dma_start(
                    y[tt * 128:(tt + 1) * 128, mb * 512:(mb + 1) * 512],
                    yt[:])

            ybp = ctx.enter_context(tc.tile_pool(name="yb", bufs=3))
            inject = []      # closures to sprinkle, one or two per slot
            for hp in range(HPC // 2):
                fc = hp
                heads = (2 * hp, 2 * hp + 1)
                # dh[qb]: denominators for query block qb; head-even on
                # partition 0, head-odd on partition 32
                dh = {qb: rcp.tile([128, 512], F32, tag=f"dh{hp}_{qb}",
                                   name=f"dh{hp}_{qb}", bufs=1)
                      for qb in range(NQB)}
                ou_all = {}
                recip_step, norm_step = make_norm_steps(hp, dh, ou_all)
                for qb in range(NQB):
                    if hp == 0 and qb == NQB - 1:
                        # hp0's qb0-2 normalize work runs during its own
                        # qb3 attention (their denominators are ready)
                        inject += [recip_step(q) for q in range(3)]
                        inject += [norm_step(h, q)
                                   for q in range(3) for h in heads]
                    nkc = 4 * (qb + 1)
                    pso = {h: psO.tile([65, 512], F32, tag=f"o{h % 2}",
                                       name=f"o{h % 2}") for h in heads}
                    ptq = {}
                    for kc in range(nkc + LAG):
                        if kc < nkc:
                            # both heads' S^T for this key chunk in one PSUM
                            # tile; the two K=64 matmuls alternate PE row
                            # groups and run concurrently
                            pss = psS.tile([128, 1024], F32, tag="s", name="s")
                            for h in heads:
                                po = 64 * (h % 2)
                                nc.tensor.matmul(
                                    pss[:, po * 8:po * 8 + 512],
                                    KT[fc][po:po + 64, kc * 128:(kc + 1) * 128],
                                    QT[fc][po:po + 64, qb * 512:(qb + 1) * 512],
                                    start=True, stop=True)
                            pt = ptp.tile([128, 1024], BF16, tag="pt", name="pt")
                            nc.scalar.activation(pt[:], pss[:], EXP)
                            if kc >= 4 * qb:  # diagonal chunk -> causal mask
                                nc.vector.tensor_tensor(
                                    pt[:], pt[:], mask_t[kc - 4 * qb][:], AL.mult)
                            ptq[kc] = pt
                        kcp = kc - LAG
                        if kcp >= 0 and kcp in ptq:
                            ptv = ptq.pop(kcp)
                            for h in heads:
                                po = 64 * (h % 2)
                                nc.tensor.matmul(
                                    pso[h][:],
                                    V[kcp][:, 65 * h:65 * h + 65],
                                    ptv[:, po * 8:po * 8 + 512],
                                    start=(kcp == 0),
                                    stop=(kcp == nkc - 1))
                        if kcp >= 0 and inject:
                            inject.pop(0)()
                            if len(inject) > 6:
                                inject.pop(0)()
                    for h in heads:
                        ou = rcp.tile([65, 512], F32,
                                      tag=f"ou{hp}_{h % 2}_{qb}",
                                      name=f"ou{hp}_{h % 2}_{qb}", bufs=1)
                        nc.vector.tensor_copy(ou[:], pso[h][:])
                        nc.sync.dma_start(
                            dh[qb][32 * (h % 2):32 * (h % 2) + 1, :],
                            ou[64:65, :])
                        ou_all[(h, qb)] = ou
                    if hp == 0 and qb == NQB - 1:
                        # qb3's normalize work drains during hp1's attention
                        inject += [recip_step(3)]
                        inject += [norm_step(h, 3) for h in heads]
                    if hp == 1:
                        if qb == 1:
                            # first-half reciprocals + qb0/1 normalize + the
                            # projection tiles they unblock, sprinkled
                            # through qb2/qb3 attention
                            inject += [recip_step(q) for q in (0, 1)]
                            inject += [norm_step(h, q)
                                       for q in (0, 1) for h in heads]
                            inject += [(lambda t=t, m=m: proj_tile(t, m, ybp))
                                       for t in range(8) for m in range(2)]
                        if qb == 2:
                            inject += [recip_step(2)]
                            inject += [norm_step(h, 2) for h in heads]
                            inject += [(lambda t=t, m=m: proj_tile(t, m, ybp))
                                       for t in range(8, 12) for m in range(2)]
                        if qb == NQB - 1:
                            inject += [recip_step(3)]
                            inject += [norm_step(h, 3) for h in heads]
                            inject += [(lambda t=t, m=m: proj_tile(t, m, ybp))
                                       for t in range(12, NKC) for m in range(2)]
            # drain any remaining injected steps (tail of the kernel)
            while inject:
                inject.pop(0)()
        ctx.close()

    nc.finalize()
    return nc


def make_masks():
    """[NQB, 128, 1024]: mask for diagonal chunk offset t, duplicated for the
    two heads (cols 0-511 and 512-1023 identical).
    keep iff query_in_block >= key_in_chunk + 128*t."""
    m = np.zeros((NQB, 128, 1024), dtype=np.float32)
    f = np.arange(512)
    p = np.arange(128)
    for t in range(NQB):
        pat = (f[None, :] >= p[:, None] + 128 * t).astype(np.float32)
        m[t][:, 0:512] = pat
        m[t][:, 512:1024] = pat
    return m


def shard_inputs(x, Wqkv, bqkv, Wproj):
    x = np.asarray(x, dtype=np.float32)
    Wqkv = np.asarray(Wqkv, dtype=np.float32)
    bqkv = np.asarray(bqkv, dtype=np.float32)
    Wproj = np.asarray(Wproj, dtype=np.float32)
    assert not np.any(bqkv[0:2048]), \
        "nonzero q/k bias not supported by the fast kernel"
    bf = ml_dtypes.bfloat16
    masks = make_masks().astype(bf)
    in_maps = []
    for c in range(8):
        b, g = c // 4, c % 4
        cs = slice(256 * g, 256 * g + 256)
        wq_ = np.ascontiguousarray(Wqkv[:, 0:1024][:, cs]) / 8.0
        wk_ = np.ascontiguousarray(Wqkv[:, 1024:2048][:, cs])
        wv_src = Wqkv[:, 2048:3072][:, cs]
        wv_ = np.zeros((DM, 260), dtype=np.float32)
        for h in range(4):
            wv_[:, 65 * h:65 * h + 64] = wv_src[:, 64 * h:64 * h + 64]
        wp_ = np.ascontiguousarray(
            Wproj[256 * g:256 * g + 256, :].reshape(2, 128, DM))
        in_maps.append({
            "xT": np.ascontiguousarray(x[b].T).astype(bf),
            "wq": wq_.astype(bf), "wk": wk_.astype(bf),
            "wv": wv_.astype(bf), "wp": wp_.astype(bf), "msk": masks,
        })
    return in_maps


def combine_outputs(results, Wqkv, bqkv, Wproj, bproj):
    bqkv = np.asarray(bqkv, dtype=np.float32)
    Wproj = np.asarray(Wproj, dtype=np.float32)
    bproj = np.asarray(bproj, dtype=np.float32)
    bv_term = bqkv[2048:3072] @ Wproj
    out = np.zeros((2, T, DM), dtype=np.float32)
    for c in range(8):
        out[c // 4] += results[c]["y"]
    out += (bv_term + bproj)[None, None, :]
    return out


_NC_CACHE = []


def _numpy_fallback(x, Wqkv, bqkv, Wproj, bproj):
    # exact-but-slow path for inputs the device kernel does not support
    b, t, dm = x.shape
    h, d = 16, 64
    qkv = x @ Wqkv + bqkv
    q, k, v = np.split(qkv, 3, axis=-1)
    q = q.reshape(b, t, h, d).transpose(0, 2, 1, 3)
    k = k.reshape(b, t, h, d).transpose(0, 2, 1, 3)
    v = v.reshape(b, t, h, d).transpose(0, 2, 1, 3)
    att = np.einsum('bhqd,bhkd->bhqk', q, k) / np.sqrt(np.float32(d))
    causal = np.tril(np.ones((t, t), dtype=bool))
    att = np.where(causal[None, None], att, -np.inf)
    att = att - att.max(axis=-1, keepdims=True)
    e = np.exp(att)
    p = e / e.sum(axis=-1, keepdims=True)
    out = np.einsum('bhqk,bhkd->bhqd', p, v)
    out = out.transpose(0, 2, 1, 3).reshape(b, t, dm)
    return (out @ Wproj + bproj).astype(np.float32)


def kernel(x, Wqkv, bqkv, Wproj, bproj):
    x = np.asarray(x, dtype=np.float32)
    Wqkv = np.asarray(Wqkv, dtype=np.float32)
    bqkv = np.asarray(bqkv, dtype=np.float32)
    Wproj = np.asarray(Wproj, dtype=np.float32)
    bproj = np.asarray(bproj, dtype=np.float32)
    if np.any(bqkv[0:2048]):
        # nonzero q/k bias falls outside the fused device kernel's contract
        return _numpy_fallback(x, Wqkv, bqkv, Wproj, bproj)
    from concourse.bass_utils import run_bass_kernel_spmd
    if not _NC_CACHE:
        _NC_CACHE.append(build_nc())
    nc = _NC_CACHE[0]
    in_maps = shard_inputs(x, Wqkv, bqkv, Wproj)
    res = run_bass_kernel_spmd(nc, in_maps, core_ids=list(range(8)))
    return combine_outputs(res.results, Wqkv, bqkv, Wproj, bproj)


# revision 10
# speedup vs baseline: 1.2788x; 1.0148x over previous
"""Trainium2 Bass kernel: multi-head self-attention (B=2, T=2048, D=1024, H=16),
8-core SPMD. Accepts FULL inputs, returns the FULL output.

Sharding: data-parallel over batch (2) x tensor-parallel over heads (4 groups
of 4). Each core computes attention for its 4 heads of one batch plus its
partial output projection; the host sums the 4 partials per batch (plus the
bias terms, folded exactly). Matmuls run in bf16 on the PE (2x the fp32r
stream rate); accumulation is fp32 in PSUM and softmax denominators stay fp32.
"""
import sys
if '/opt/trn_rl_repo' not in sys.path:
    sys.path.insert(0, '/opt/trn_rl_repo')
import numpy as np
import ml_dtypes
import concourse.bass as bass
import concourse.mybir as mybir
from concourse import bacc
from concourse.tile import TileContext

F32 = mybir.dt.float32
F32R = mybir.dt.float32r
BF16 = mybir.dt.bfloat16
AL = mybir.AluOpType
EXP = mybir.ActivationFunctionType.Exp

T = 2048
DM = 1024
HPC = 4
D = 64
NQB = 4           # query blocks of 512
NKC = 16          # key chunks of 128
NDC = 8           # contraction chunks of 128 for projections
LAG = 3           # PV lags S/exp by this many key chunks


def build_nc():
    nc = bacc.Bacc("TRN2", target_bir_lowering=False, debug=True)

    xT = nc.dram_tensor("xT", [DM, T], BF16, kind="ExternalInput")
    wq = nc.dram_tensor("wq", [DM, 256], BF16, kind="ExternalInput")
    wk = nc.dram_tensor("wk", [DM, 256], BF16, kind="ExternalInput")
    wv = nc.dram_tensor("wv", [DM, 260], BF16, kind="ExternalInput")
    wp = nc.dram_tensor("wp", [2, 128, DM], BF16, kind="ExternalInput")
    msk = nc.dram_tensor("msk", [NQB, 128, 1024], BF16, kind="ExternalInput")
    y = nc.dram_tensor("y", [T, DM], BF16, kind="ExternalOutput")

    with nc.allow_low_precision("bf16 matmul pipeline"), TileContext(nc) as tc:
        from contextlib import ExitStack
        ctx = ExitStack()
        cp = ctx.enter_context(tc.tile_pool(name="const", bufs=1))
        wtp = ctx.enter_context(tc.tile_pool(name="wts", bufs=1))
        qkvp = ctx.enter_context(tc.tile_pool(name="qkv", bufs=1))
        psS = ctx.enter_context(tc.tile_pool(name="psS", bufs=2, space="PSUM"))
        psO = ctx.enter_context(tc.tile_pool(name="psO", bufs=1, space="PSUM"))
        psX = ctx.enter_context(tc.tile_pool(name="psX", bufs=2, space="PSUM"))

        # constants: duplicated causal masks (same pattern for both heads)
        # (mask DMAs are emitted after the xT/weight DMAs below - phase C
        # needs them much later)
        mask_t = [cp.tile([128, 1024], BF16, tag=f"m{i}", name=f"m{i}")
                  for i in range(NQB)]
        ones32 = cp.tile([128, 64], F32, tag="ones32", name="ones32")
        nc.vector.memset(ones32[:], 1.0)
        # preload the exp ACT table set while the input DMAs stream in
        warm = cp.tile([1, 8], F32, tag="warm", name="warm")
        nc.scalar.activation(warm[:], ones32[0:1, 0:8], EXP)
        ones_t = cp.tile([128, 64], BF16, tag="ones", name="ones")
        nc.vector.tensor_copy(ones_t[:], ones32[:])
        onesv = cp.tile([128, 4], F32, tag="onesv", name="onesv")
        nc.vector.memset(onesv[:], 1.0)
        # PE warm-up: ~4.5us of dummy matmuls with no DMA deps flips the
        # HAM clock gate to 8/8 (2.4 GHz) before the real streams begin
        wmt = cp.tile([128, 512], BF16, tag="wmt", name="wmt")
        nc.vector.memset(wmt[:], 0.0)
        psW = psX.tile([128, 512], F32, tag="b", name="b")
        for i in range(12):
            nc.tensor.matmul(psW[:], wmt[:, 0:128], wmt[:],
                             start=(i == 0), stop=(i == 11))

        # weights
        wq_t = [wtp.tile([128, 256], BF16, tag=f"wq{k}", name=f"wq{k}")
                for k in range(NDC)]
        wk_t = [wtp.tile([128, 256], BF16, tag=f"wk{k}", name=f"wk{k}")
                for k in range(NDC)]
        wv_t = [wtp.tile([128, 260], BF16, tag=f"wv{k}", name=f"wv{k}")
                for k in range(NDC)]
        wp_t = [wtp.tile([128, DM], BF16, tag=f"wp{j}", name=f"wp{j}")
                for j in range(2)]

        # persistent activations
        QT = [qkvp.tile([128, T], BF16, tag=f"qt{i}", name=f"qt{i}") for i in range(2)]
        KT = [qkvp.tile([128, T], BF16, tag=f"kt{i}", name=f"kt{i}") for i in range(2)]
        V = [qkvp.tile([128, 260], BF16, tag=f"v{t}", name=f"v{t}") for t in range(NKC)]
        # stacked O^T: feature-chunk jc holds heads (2jc, 2jc+1) on
        # partitions 0-63 / 64-127
        OTS = [qkvp.tile([128, T], BF16, tag=f"ots{j}", name=f"ots{j}")
               for j in range(2)]

        # ---------------- phase B: QKV projections ----------------
        with tc.tile_pool(name="xt", bufs=1) as xtp:
            xt_t = [xtp.tile([128, T], BF16, tag=f"x{k}", name=f"x{k}")
                    for k in range(NDC)]
            # DMA order matches consumption: Q needs wq + x cols 0-1023
            # first (tbp=0), then the second x halves (tbp=1), then wk, ...
            for k in range(NDC):
                nc.sync.dma_start(wq_t[k][:], wq[k * 128:(k + 1) * 128, :])
                nc.sync.dma_start(xt_t[k][:, 0:1024],
                                  xT[k * 128:(k + 1) * 128, 0:1024])
            for k in range(NDC):
                nc.sync.dma_start(xt_t[k][:, 1024:2048],
                                  xT[k * 128:(k + 1) * 128, 1024:2048])
            for k in range(NDC):
                nc.sync.dma_start(wk_t[k][:], wk[k * 128:(k + 1) * 128, :])
            for k in range(NDC):
                nc.sync.dma_start(wv_t[k][:], wv[k * 128:(k + 1) * 128, :])
            for i in range(NQB):
                nc.sync.dma_start(mask_t[i][:], msk[i])
            for j in range(2):
                nc.sync.dma_start(wp_t[j][:], wp[j])
            ncopy = 0
            for W, OUT in ((wq_t, QT), (wk_t, KT)):
                for tbp in range(2):
                    for fc in range(2):
                        ps = psS.tile([128, 1024], F32, tag="s", name="s")
                        for j in (0, 1):
                            tb = 2 * tbp + j
                            for k in range(NDC):
                                nc.tensor.matmul(
                                    ps[:, j * 512:(j + 1) * 512],
                                    W[k][:, fc * 128:(fc + 1) * 128],
                                    xt_t[k][:, tb * 512:(tb + 1) * 512],
                                    start=(k == 0), stop=(k == NDC - 1))
                        dst = OUT[fc][:, tbp * 1024:(tbp + 1) * 1024]
                        if ncopy % 2 == 0:
                            nc.scalar.copy(dst, ps[:])
                        else:
                            nc.vector.tensor_copy(dst, ps[:])
                        ncopy += 1
            for tt in range(NKC):
                ps = psX.tile([128, 260], F32, tag="b", name="b")
                for k in range(NDC):
                    nc.tensor.matmul(
                        ps[:], xt_t[k][:, tt * 128:(tt + 1) * 128], wv_t[k][:],
                        start=(k == 0), stop=(k == NDC - 1))
                if tt % 2 == 0:
                    nc.scalar.copy(V[tt][:], ps[:])
                else:
                    nc.vector.tensor_copy(V[tt][:], ps[:])
                nc.vector.tensor_copy(
                    V[tt].rearrange("p (h c) -> p h c", c=65)[:, :, 64:65],
                    onesv[:].rearrange("p (h c) -> p h c", c=1))

        # ---------------- phase C: attention ----------------
        ptp = ctx.enter_context(tc.tile_pool(name="pt", bufs=5))
        rcp = ctx.enter_context(tc.tile_pool(name="rcp", bufs=2))
        if True:

            def make_norm_steps(hp, dh, ou_all):
                """Closures: recip_step(qb) inverts the [dh] row pair for one
                query block (head-even at partition 0, head-odd at 32);
                norm_step(h, qb) does the K=1 broadcast matmul + normalize
                multiply. Callers sprinkle these through dense K=128 matmul
                streams so HAM row activity stays high."""
                dhrr = {}

                def recip_step(qb):
                    def emit():
                        dhr = rcp.tile([64, 512], F32, tag="dhr",
                                       name="dhr", bufs=2)
                        nc.vector.reciprocal_approx_fast(dhr[:], dh[qb][0:64, :])
                        dd = rcp.tile([64, 512], BF16, tag="dhrr",
                                      name="dhrr", bufs=2)
                        nc.vector.tensor_copy(dd[:], dhr[:])
                        dhrr[qb] = dd
                    return emit

                def norm_step(h, qb):
                    def emit():
                        hh = h % 2
                        r = 32 * hh
                        dd = dhrr[qb]
                        # bf16 broadcast matmul: ones exact, recip ~0.2% rms
                        psb = psX.tile([64, 512], F32, tag="b", name="b")
                        nc.tensor.matmul(
                            psb[:], ones_t[r:r + 1, 0:64], dd[r:r + 1, :],
                            start=True, stop=True)
                        if hh == 0:
                            nc.vector.tensor_tensor(
                                OTS[hp][0:64, qb * 512:(qb + 1) * 512],
                                ou_all[(h, qb)][0:64, :], psb[:], AL.mult)
                        else:
                            # odd head: normalize to a bounce tile, DMA to
                            # partitions 64-127 of the stacked O^T
                            ob = rcp.tile([64, 512], BF16, tag="ob",
                                          name="ob", bufs=2)
                            nc.vector.tensor_tensor(
                                ob[:], ou_all[(h, qb)][0:64, :], psb[:],
                                AL.mult)
                            nc.sync.dma_start(
                                OTS[hp][64:128, qb * 512:(qb + 1) * 512],
                                ob[:])
                    return emit

                return recip_step, norm_step

            def proj_tile(tt, mb, ybp):
                psy = psX.tile([128, 512], F32, tag="b", name="yps")
                for jc in range(2):
                    nc.tensor.matmul(
                        psy[:],
                        OTS[jc][:, tt * 128:(tt + 1) * 128],
                        wp_t[jc][:, mb * 512:(mb + 1) * 512],
                        start=(jc == 0), stop=(jc == 1))
                yt = ybp.tile([128, 512], BF16, tag="yt", name="yt")
                # keep proj copies off the scalar engine: attention is
                # exp(scalar)-bound once the matmuls are bf16
                nc.vector.tensor_copy(yt[:], psy[:])
                nc.sync.dma_start(
                    y[tt * 128:(tt + 1) * 128, mb * 512:(mb + 1) * 512],
                    yt[:])

            ybp = ctx.enter_context(tc.tile_pool(name="yb", bufs=3))
            inject = []      # closures to sprinkle, one or two per slot
            for hp in range(HPC // 2):
                fc = hp
                heads = (2 * hp, 2 * hp + 1)
                # dh[qb]: denominators for query block qb; head-even on
                # partition 0, head-odd on partition 32
                dh = {qb: rcp.tile([128, 512], F32, tag=f"dh{hp}_{qb}",
                                   name=f"dh{hp}_{qb}", bufs=1)
                      for qb in range(NQB)}
                ou_all = {}
                recip_step, norm_step = make_norm_steps(hp, dh, ou_all)
                for qb in range(NQB):
                    if hp == 0 and qb == NQB - 1:
                        # hp0's qb0-2 normalize work runs during its own
                        # qb3 attention (their denominators are ready)
                        inject += [recip_step(q) for q in range(3)]
                        inject += [norm_step(h, q)
                                   for q in range(3) for h in heads]
                    nkc = 4 * (qb + 1)
                    pso = {h: psO.tile([65, 512], F32, tag=f"o{h % 2}",
                                       name=f"o{h % 2}") for h in heads}
                    ptq = {}
                    for kc in range(nkc + LAG):
                        if kc < nkc:
                            # both heads' S^T for this key chunk in one PSUM
                            # tile; the two K=64 matmuls alternate PE row
                            # groups and run concurrently
                            pss = psS.tile([128, 1024], F32, tag="s", name="s")
                            for h in heads:
                                po = 64 * (h % 2)
                                nc.tensor.matmul(
                                    pss[:, po * 8:po * 8 + 512],
                                    KT[fc][po:po + 64, kc * 128:(kc + 1) * 128],
                                    QT[fc][po:po + 64, qb * 512:(qb + 1) * 512],
                                    start=True, stop=True)
                            pt = ptp.tile([128, 1024], BF16, tag="pt", name="pt")
                            nc.scalar.activation(pt[:], pss[:], EXP)
                            if kc >= 4 * qb:  # diagonal chunk -> causal mask
                                nc.vector.tensor_tensor(
                                    pt[:], pt[:], mask_t[kc - 4 * qb][:], AL.mult)
                            ptq[kc] = pt
                        kcp = kc - LAG
                        if kcp >= 0 and kcp in ptq:
                            ptv = ptq.pop(kcp)
                            for h in heads:
                                po = 64 * (h % 2)
                                nc.tensor.matmul(
                                    pso[h][:],
                                    V[kcp][:, 65 * h:65 * h + 65],
                                    ptv[:, po * 8:po * 8 + 512],
                                    start=(kcp == 0),
                                    stop=(kcp == nkc - 1))
                        if kcp >= 0 and inject:
                            inject.pop(0)()
                            if len(inject) > 6:
                                inject.pop(0)()
                    for h in heads:
                        ou = rcp.tile([65, 512], F32,
                                      tag=f"ou{hp}_{h % 2}_{qb}",
                                      name=f"ou{hp}_{h % 2}_{qb}", bufs=1)
                        nc.vector.tensor_copy(ou[:], pso[h][:])
                        nc.sync.dma_start(
                            dh[qb][32 * (h % 2):32 * (h % 2) + 1, :],
                            ou[64:65, :])
                        ou_all[(h, qb)] = ou
                    if hp == 0 and qb == NQB - 1:
                        # qb3's normalize work drains during hp1's attention
                        inject += [recip_step(3)]
                        inject += [norm_step(h, 3) for h in heads]
                    if hp == 1:
                        if qb == 1:
                            # first-half reciprocals + qb0/1 normalize + the
                            # projection tiles they unblock, sprinkled
                            # through qb2/qb3 attention
                            inject += [recip_step(q) for q in (0, 1)]
                            inject += [norm_step(h, q)
                                       for q in (0, 1) for h in heads]
                            inject += [(lambda t=t, m=m: proj_tile(t, m, ybp))
                                       for t in range(8) for m in range(2)]
                        if qb == 2:
                            inject += [recip_step(2)]
                            inject += [norm_step(h, 2) for h in heads]
                            inject += [(lambda t=t, m=m: proj_tile(t, m, ybp))
                                       for t in range(8, 12) for m in range(2)]
                        if qb == NQB - 1:
                            inject += [recip_step(3)]
                            inject += [norm_step(h, 3) for h in heads]
                            inject += [(lambda t=t, m=m: proj_tile(t, m, ybp))
                                       for t in range(12, NKC) for m in range(2)]
            # drain any remaining injected steps (tail of the kernel)
            while inject:
                inject.pop(0)()
        ctx.close()

    nc.finalize()
    return nc


def make_masks():
    """[NQB, 128, 1024]: mask for diagonal chunk offset t, duplicated for the
    two heads (cols 0-511 and 512-1023 identical).
    keep iff query_in_block >= key_in_chunk + 128*t."""
    m = np.zeros((NQB, 128, 1024), dtype=np.float32)
    f = np.arange(512)
    p = np.arange(128)
    for t in range(NQB):
        pat = (f[None, :] >= p[:, None] + 128 * t).astype(np.float32)
        m[t][:, 0:512] = pat
        m[t][:, 512:1024] = pat
    return m


def shard_inputs(x, Wqkv, bqkv, Wproj):
    x = np.asarray(x, dtype=np.float32)
    Wqkv = np.asarray(Wqkv, dtype=np.float32)
    bqkv = np.asarray(bqkv, dtype=np.float32)
    Wproj = np.asarray(Wproj, dtype=np.float32)
    assert not np.any(bqkv[0:2048]), \
        "nonzero q/k bias not supported by the fast kernel"
    bf = ml_dtypes.bfloat16
    masks = make_masks().astype(bf)
    in_maps = []
    for c in range(8):
        b, g = c // 4, c % 4
        cs = slice(256 * g, 256 * g + 256)
        wq_ = np.ascontiguousarray(Wqkv[:, 0:1024][:, cs]) / 8.0
        wk_ = np.ascontiguousarray(Wqkv[:, 1024:2048][:, cs])
        wv_src = Wqkv[:, 2048:3072][:, cs]
        wv_ = np.zeros((DM, 260), dtype=np.float32)
        for h in range(4):
            wv_[:, 65 * h:65 * h + 64] = wv_src[:, 64 * h:64 * h + 64]
        wp_ = np.ascontiguousarray(
            Wproj[256 * g:256 * g + 256, :].reshape(2, 128, DM))
        in_maps.append({
            "xT": np.ascontiguousarray(x[b].T).astype(bf),
            "wq": wq_.astype(bf), "wk": wk_.astype(bf),
            "wv": wv_.astype(bf), "wp": wp_.astype(bf), "msk": masks,
        })
    return in_maps


def combine_outputs(results, Wqkv, bqkv, Wproj, bproj):
    bqkv = np.asarray(bqkv, dtype=np.float32)
    Wproj = np.asarray(Wproj, dtype=np.float32)
    bproj = np.asarray(bproj, dtype=np.float32)
    bv_term = bqkv[2048:3072] @ Wproj
    out = np.zeros((2, T, DM), dtype=np.float32)
    for c in range(8):
        out[c // 4] += results[c]["y"].astype(np.float32)
    out += (bv_term + bproj)[None, None, :]
    return out


_NC_CACHE = []


def _numpy_fallback(x, Wqkv, bqkv, Wproj, bproj):
    # exact-but-slow path for inputs the device kernel does not support
    b, t, dm = x.shape
    h, d = 16, 64
    qkv = x @ Wqkv + bqkv
    q, k, v = np.split(qkv, 3, axis=-1)
    q = q.reshape(b, t, h, d).transpose(0, 2, 1, 3)
    k = k.reshape(b, t, h, d).transpose(0, 2, 1, 3)
    v = v.reshape(b, t, h, d).transpose(0, 2, 1, 3)
    att = np.einsum('bhqd,bhkd->bhqk', q, k) / np.sqrt(np.float32(d))
    causal = np.tril(np.ones((t, t), dtype=bool))
    att = np.where(causal[None, None], att, -np.inf)
    att = att - att.max(axis=-1, keepdims=True)
    e = np.exp(att)
    p = e / e.sum(axis=-1, keepdims=True)
    out = np.einsum('bhqk,bhkd->bhqd', p, v)
    out = out.transpose(0, 2, 1, 3).reshape(b, t, dm)
    return (out @ Wproj + bproj).astype(np.float32)


def kernel(x, Wqkv, bqkv, Wproj, bproj):
    x = np.asarray(x, dtype=np.float32)
    Wqkv = np.asarray(Wqkv, dtype=np.float32)
    bqkv = np.asarray(bqkv, dtype=np.float32)
    Wproj = np.asarray(Wproj, dtype=np.float32)
    bproj = np.asarray(bproj, dtype=np.float32)
    if np.any(bqkv[0:2048]):
        # nonzero q/k bias falls outside the fused device kernel's contract
        return _numpy_fallback(x, Wqkv, bqkv, Wproj, bproj)
    from concourse.bass_utils import run_bass_kernel_spmd
    if not _NC_CACHE:
        _NC_CACHE.append(build_nc())
    nc = _NC_CACHE[0]
    in_maps = shard_inputs(x, Wqkv, bqkv, Wproj)
    res = run_bass_kernel_spmd(nc, in_maps, core_ids=list(range(8)))
    return combine_outputs(res.results, Wqkv, bqkv, Wproj, bproj)
